# revision 24
# baseline (speedup 1.0000x reference)
"""Trainium2 Bass kernel for nn_Concat_26147760898611.

Mean-pool over the word dim of article_concat [256, 2048, 300] and
options_concat [256, 64, 300], concat features -> [256, 600].

Sharding: pure data parallel over batch across 8 NeuronCores
(32 batches per core). Per core ~81 MB must stream from HBM; each of
the 16 SDMA engines caps at ~26.6 GB/s on its SBUF AXI port (measured;
f32->bf16 cast-DMA does NOT beat it -- the engine is input-bound), so
the 190us stream at ~425 GB/s is the floor and everything else is
built to keep head/tail overhead off that stream:

  - article batches 0..28 are single 2.46 MB transfers into SBUF tiles
    [128 partitions, 16 words, 300 feat] (19.2 KB contiguous per
    partition; HWDGE assigns line i of a transfer to engine i mod 16,
    so a 128-line transfer stripes all 16 engines evenly).
  - options ride the Scalar HWDGE ring while article b0/b1 open the
    Sync ring: both rings generate descriptors in parallel at the head.
  - folds halve the word axis and convert to bf16; even batches fold
    twice on DVE (4 matmuls), odd batches once on GpSimd (8 matmuls;
    GpSimd's fold is ~2x slower, a second level would not keep up).
  - the folded word columns are reduced across the partition dim by
    single-pass bf16 matmuls with a selector whose single nonzero
    column is 1/n_words (exact in bf16), routing batch b's MEAN into
    PSUM row b; rel err ~2e-3 vs the 2e-2 gate.
  - psum_a1 accumulates batches 0..28 and stops ~17us before the
    stream ends, so the early drain (rows 0..28: ACT copy + 29-line
    store + its ~2.5us HBM-write receipt) hides under the stream even
    when DMA-completion semaphores lag a contended stream.
  - the last three batches stream as word-chunk DMAs ([8,8], [8,8],
    [8,4,2,2] words) with separate completion semaphores into psum_a2;
    every chunk is a light DVE double-fold plus <=2 matmuls, so after
    the final 2-word chunk lands only ~0.5us fold + 1 matmul + ACT copy
    + a 3-line store remain: tail is ~6.5us vs ~12.6us for a monolithic
    last batch (chunk sems lag their last byte by 0.4..2.5us).
  - PSUM drains stay on Scalar ACTIVATE (a DVE tensor_copy issued
    ~40ns after the stop matmul intermittently read stale PSUM).
  - stores ride the Scalar ring: there their lines round-robin across
    engines; on the Sync ring an HBM-contiguous store fuses into one
    descriptor pinned to a single engine, stretching its FIFO.
  - out_t2 is a separate tile for the final rows so the tail copy
    never write-after-read waits on the early-drain store.
  - sel_a is built on-chip with memsets; sel_o (block-diagonal, 1/64)
    comes in via a small DMA since compute memsets must start on a
    quadrant partition. 12 warmup matmuls flip the HAM clock gate up
    before real data lands.

Known residual variance: engine 15 (E79) runs up to +20% slower per
byte when a neighbor tenant contends; sub-128-line transfers always
restart engine assignment at engine 0, so load cannot be shifted off
E79 without overloading engines 0..7 by the same amount -- measured
range 206-250us is stream contention, not kernel structure.

Self-contained: hardcodes all shapes; no file reads.
"""

import numpy as np

N_CORES = 8
B = 256  # full batch
BC = B // N_CORES  # 32 batches per core
DIM = 300
AW = 2048  # article words per batch
OW = 64  # options words per batch
P = 128  # SBUF partitions
AWP = AW // P  # 16 article words per partition
OWP = 16  # options words per partition (4 batches x 16 words)
DATA_BUFS = 6
FOLD_BUFS = 3
WARMUP_MMS = 12

_CACHE = {}


def _build_nc():
    import concourse.bacc as bacc
    import concourse.mybir as mybir
    import concourse.tile as tile

    f32 = mybir.dt.float32
    bf16 = mybir.dt.bfloat16
    nc = bacc.Bacc("TRN2", target_bir_lowering=False, debug=False)

    art = nc.dram_tensor("article", [BC, AW, DIM], f32, kind="ExternalInput")
    opt = nc.dram_tensor("options", [BC, OW, DIM], f32, kind="ExternalInput")
    sel_o = nc.dram_tensor("sel_o", [P, BC], bf16, kind="ExternalInput")
    out = nc.dram_tensor("out", [BC, 2 * DIM], f32, kind="ExternalOutput")

    # [BC, 128, 16, 300]: partition p <- words p*16 .. p*16+15 (contiguous)
    art_r = art.ap().rearrange("b (p w) f -> b p w f", p=P)
    # [128, 16, 300]: partition p <- 16 consecutive words of batch p//4
    opt_r = opt.ap().rearrange("b (s q) f -> (b s) q f", s=P // BC)

    with tile.TileContext(nc) as tc:
        with (
            tc.tile_pool(name="const", bufs=1) as cpool,
            tc.tile_pool(name="data", bufs=DATA_BUFS) as dpool,
            tc.tile_pool(name="fold", bufs=FOLD_BUFS) as fpool,
            tc.tile_pool(name="outp", bufs=1) as opool,
            tc.tile_pool(name="psum", bufs=1, space="PSUM") as ppool,
        ):
            # sel_o first on the Scalar ring (tiny, needed by the PE warmup),
            # then options on the same ring while article b0/b1 go on the
            # Sync ring -- both HWDGE rings generate descriptors in parallel
            # so the stream starts as early as the queues allow
            sel_o_t = cpool.tile([P, BC], bf16, tag="sel_o")
            nc.scalar.dma_start(sel_o_t[:], sel_o.ap()[:])
            t_opt = dpool.tile([P, OWP, DIM], f32, tag="data")
            nc.scalar.dma_start(t_opt[:], opt_r[:])
            pre = []
            for b in range(2):
                t = dpool.tile([P, AWP, DIM], f32, tag="data")
                if b == 0:
                    # first transfer split 16+112 lines: the 16-line piece
                    # needs far fewer descriptors generated before its
                    # doorbell, so all 16 engines get a first packet sooner
                    nc.sync.dma_start(t[0:16], art_r[b][0:16])
                    nc.sync.dma_start(t[16:P], art_r[b][16:P])
                else:
                    nc.sync.dma_start(t[:], art_r[b])
                pre.append(t)

            # sel_a built on-chip: a zero band whose single all-ones
            # column carries 1/AW (exact in bf16), so PSUM accumulates
            # the mean directly. sel_o (block-diagonal, 1/OW) comes in
            # via a small DMA since compute memsets must start on a
            # quadrant partition.
            sel_a_t = cpool.tile([P, 2 * BC - 1], bf16, tag="sel_a")
            nc.gpsimd.memset(sel_a_t[:], 0.0)
            nc.gpsimd.memset(sel_a_t[:, BC - 1 : BC], 1.0 / AW)

            psum_a1 = ppool.tile([BC, DIM], f32, tag="psum_a1")
            psum_a2 = ppool.tile([BC, DIM], f32, tag="psum_a2")
            psum_b = ppool.tile([BC, DIM], f32, tag="psum_b")
            psum_w = ppool.tile([BC, 2 * BC - 1], f32, tag="psum_w")

            # PE warmup: flip the HAM clock gate up before real data lands
            for _ in range(WARMUP_MMS):
                nc.tensor.matmul(
                    psum_w[:], sel_o_t[:], sel_a_t[:], start=True, stop=True
                )

            def fold_and_mm(t, nch, sel_ap, psum, first, last, eng, ftag,
                            twice=False):
                # halve the word axis once (f32 -> bf16), then one bf16
                # matmul per surviving word column; DVE batches fold a
                # second time (bf16, cheap) to halve their PE matmul count
                n = nch // 2
                f = fpool.tile([P, n, DIM], bf16, tag=ftag)
                eng.tensor_add(f[:], t[:, 0:n, :], t[:, n : 2 * n, :])
                if twice:
                    n2 = n // 2
                    f2 = fpool.tile([P, n2, DIM], bf16, tag=ftag + "b")
                    eng.tensor_add(f2[:], f[:, 0:n2, :], f[:, n2 : 2 * n2, :])
                    f, n = f2, n2
                for j in range(n):
                    nc.tensor.matmul(
                        psum[:],
                        sel_ap,
                        f[:, j, :],
                        start=(first and j == 0),
                        stop=(last and j == n - 1),
                    )

            out_t = opool.tile([BC, 2 * DIM], f32, tag="out")
            # separate tile for the final row so the tail PSUM copy never
            # write-after-read conflicts with the early-drain store of out_t
            out_t2 = opool.tile([BC, 2 * DIM], f32, tag="out2")

            # dummy ACTIVATE right after warmup: pulls the Scalar act
            # tables through E64 during the DMA-idle preamble instead of
            # mid-stream, where the ~11KB load displaces stream packets
            # on E64's FIFO
            scratch_t = opool.tile([BC, 2 * BC - 1], f32, tag="scratch")
            nc.scalar.copy(scratch_t[:], psum_w[:])

            # options: fold + 8 matmuls -> psum_b holds the means
            # (PSUM can't source a DMA, so drains are plain copies)
            fold_and_mm(t_opt, OWP, sel_o_t[:], psum_b, True, True,
                        nc.vector, "fold8")
            # PSUM drains stay on Scalar ACTIVATE: its semaphore chain
            # provably waits for the PE accumulation drain, where a DVE
            # tensor_copy issued ~40ns after the stop matmul intermittently
            # read stale PSUM (rel err 2e-1 once in ~6 runs)
            nc.scalar.copy(out_t[:, DIM : 2 * DIM], psum_b[:])
            nc.scalar.copy(out_t2[:, DIM : 2 * DIM], psum_b[:])

            # article batches: 0..28 accumulate in psum_a1 (its stop fires
            # ~3 batches before the stream ends, so the early drain's
            # single-engine ~2us store + HBM receipt hide fully under the
            # stream); 29 and 30 join batch 31's chunks in psum_a2. Even
            # batches fold on DVE (twice: 4 matmuls), odd on GpSimd (once:
            # 8 matmuls -- GpSimd's fold is ~2x slower, a second level
            # would not keep up); b=28 lands on DVE so the last full fold
            # before the chunked batches is the fast one
            # psum_a1 takes batches 0..A1N-1; its stop fires ~5 batches
            # before the stream ends so the early drain (copy + store +
            # ~2.5us HBM receipt) hides under the stream even when a
            # contended stream delays completion semaphores by ~3us.
            # Batches A1N..28 are ordinary full batches that join psum_a2
            # ahead of the final chunk matmuls.
            A1N = BC - 5  # batches 0..26 -> psum_a1; 27..31 -> psum_a2
            for b in range(BC - 3):
                if b < 2:
                    t = pre[b]
                else:
                    t = dpool.tile([P, AWP, DIM], f32, tag="data")
                    nc.sync.dma_start(t[:], art_r[b])
                eng = nc.vector if b % 2 == 0 else nc.gpsimd
                fold_and_mm(
                    t,
                    AWP,
                    sel_a_t[:, BC - 1 - b : 2 * BC - 1 - b],
                    psum_a1 if b < A1N else psum_a2,
                    b == 0 or b == A1N,
                    b == A1N - 1,
                    eng,
                    "fold8",
                    twice=(b % 2 == 0),
                )

            # last three batches into psum_a2, streamed as word-chunk DMAs
            # with separate completion semaphores. Every chunk gets a light
            # DVE double-fold and at most 2 matmuls, so no heavy fold or
            # long matmul run is ever gated by a late completion semaphore
            # near the stream end; after the final 2-word chunk lands only
            # one tiny fold + one matmul remain before the stop
            chunk_plan = [
                (BC - 3, [8, 8]),
                (BC - 2, [8, 8]),
                (BC - 1, [8, 4, 2, 2]),
            ]
            MM_PER = {8: 2, 4: 1, 2: 1}
            ch_tiles = []
            for b, chunks in chunk_plan:
                w0 = 0
                for cw in chunks:
                    tl = dpool.tile(
                        [P, cw, DIM], f32, tag=f"ch{cw}", bufs=3 if cw == 8 else 2
                    )
                    nc.sync.dma_start(tl[:], art_r[b][:, w0 : w0 + cw, :])
                    ch_tiles.append(tl)
                    w0 += cw
            n_ch_mm = sum(MM_PER[cw] for _, chs in chunk_plan for cw in chs)
            nmm = 0
            ti = 0
            for b, chunks in chunk_plan:
                sel_b = sel_a_t[:, BC - 1 - b : 2 * BC - 1 - b]
                for cw in chunks:
                    tl = ch_tiles[ti]
                    ti += 1
                    h = cw // 2
                    fh = fpool.tile(
                        [P, h, DIM], bf16, tag=f"cf{h}", bufs=2
                    )
                    nc.vector.tensor_add(
                        fh[:], tl[:, 0:h, :], tl[:, h : 2 * h, :]
                    )
                    if h >= 2:
                        h2 = h // 2
                        fh2 = fpool.tile(
                            [P, h2, DIM], bf16, tag=f"cf{h}b", bufs=2
                        )
                        nc.vector.tensor_add(
                            fh2[:], fh[:, 0:h2, :], fh[:, h2 : 2 * h2, :]
                        )
                        fh, h = fh2, h2
                    for j in range(h):
                        nc.tensor.matmul(
                            psum_a2[:],
                            sel_b,
                            fh[:, j, :],
                            start=False,  # psum_a2 group opened by b==A1N
                            stop=(nmm == n_ch_mm - 1),
                        )
                        nmm += 1

            # early drain: rows 0..28 (both halves) stored while the last
            # batches stream. The store rides the Scalar HWDGE ring: there
            # its 29 lines round-robin across SDMA engines, while on the
            # Sync ring they fuse into one descriptor pinned to E64,
            # stretching E64's FIFO right when the final chunks need it
            nc.scalar.copy(out_t[0:A1N, 0:DIM], psum_a1[0:A1N, :])
            nc.scalar.dma_start(out.ap()[0:A1N, :], out_t[0:A1N, :])

            # tail drain: copy all of psum_a2 into out_t2's article columns
            # (copies must start at partition 0; out_t2 is untouched by the
            # early-drain store, so no write-after-read wait), then rows
            # 29..31 go out as one contiguous 7.2KB store.
            nc.scalar.copy(out_t2[:, 0:DIM], psum_a2[:])
            nc.scalar.dma_start(out.ap()[A1N:BC, :], out_t2[A1N:BC, :])

    nc.compile()
    return nc


def get_nc():
    if "nc" not in _CACHE:
        _CACHE["nc"] = _build_nc()
    return _CACHE["nc"]


def make_in_maps(article, options):
    import ml_dtypes

    article = np.ascontiguousarray(np.asarray(article, dtype=np.float32))
    options = np.ascontiguousarray(np.asarray(options, dtype=np.float32))
    assert article.shape == (B, AW, DIM), article.shape
    assert options.shape == (B, OW, DIM), options.shape
    sel_o = np.zeros((P, BC), np.float32)
    sel_o[np.arange(P), np.arange(P) // (P // BC)] = 1.0 / OW
    sel_o = sel_o.astype(ml_dtypes.bfloat16)
    return [
        {
            "article": article[i * BC : (i + 1) * BC],
            "options": options[i * BC : (i + 1) * BC],
            "sel_o": sel_o,
        }
        for i in range(N_CORES)
    ]


def run_sharded(article, options, **spmd_kwargs):
    from concourse.bass_utils import run_bass_kernel_spmd

    nc = get_nc()
    in_maps = make_in_maps(article, options)
    res = run_bass_kernel_spmd(nc, in_maps, list(range(N_CORES)), **spmd_kwargs)
    full = np.concatenate(
        [res.results[i]["out"] for i in range(N_CORES)], axis=0
    ).astype(np.float32)
    return full, res


def kernel(article_concat, options_concat):
    full, _ = run_sharded(article_concat, options_concat)
    return full

